# revision 33
# baseline (speedup 1.0000x reference)
"""Causal attention kernel for Trainium2 (Bass/Tile), 8-core data parallel.

Problem: B=16, L=2048, D=1024 fp32.
    scores = q @ k^T  (per batch), causal additive mask (-1e10), softmax
    over keys with scale sqrt(1024)=32, out = probs @ v.

Sharding: batch dim across the 8 cores (2 batches per core), no
cross-core comms. Each core runs an identical program (SPMD).

Per-core scheme (per batch, matmuls fp16 in / fp32 PSUM; optionally the
last 256*ndr contraction dims of QK^T run as fp8-e4m3 DoubleRow pairs):
  - Host supplies q and k pre-transposed to [D, L] fp16 so the
    contraction dim D sits on SBUF partitions; v stays [L, D] fp16.
  - S^T tiles [128 k, 512 q] = K^T_chunk.T @ Q^T_chunk accumulated over
    d-chunks in PSUM. Off-diagonal k-tiles run first (they use K groups
    already resident from earlier chunks, hiding the fresh K chunk DMA;
    measured ~10us/iter better than diagonal-first).
  - P^T = exp((S^T + mask)/32) on ScalarE -> fp16 SBUF tiles. Fully
    masked column ranges are never computed (memset 0); the diagonal
    128x128 sub-block gets a 0/-1e10 additive mask tile in PSUM.
  - O tile [128 q, 1024] = sum_kt P^T[kt].T @ V[kt] accumulated in PSUM.
    Softmax denominators ride the same sweep as N=1 matmuls
    (lhsT = the PV weights, rhs = ones) accumulating [128 q, 1] in
    PSUM -- per-partition sums that feed a DVE reciprocal directly, so
    the old [1,512] row-sum matmuls + transpose dance disappear.
  - Normalize with per-partition scalar multiply, DMA out fp16.
"""

import numpy as np

import concourse.bass as bass
import concourse.mybir as mybir
import concourse.tile as tile
from concourse.bass_utils import run_bass_kernel_spmd
from concourse.tile import ScopedClock

F32 = mybir.dt.float32
F16 = mybir.dt.float16
F8 = mybir.dt.float8e4
DRMODE = mybir.MatmulPerfMode.DoubleRow

N_CORES = 8
BPC = 2  # batches per core
L = 2048
D = 1024
P = 128
NQS = L // 512  # 4 q-chunks of 512
MASK_VAL = -1.0e10
SCALE = 1.0 / 32.0

NDR = 1  # DoubleRow fp8 d-steps (256 dims each) used for QK; rest fp16


def _patched_drain_and_barrier(self, tick_clock, wait_clock):
    """Workaround for walrus 'Too many sync wait commands' on the Tile exit
    Drain: re-emit the global-clock sem waits as standalone SP NoOps (one
    wait each) before the drain, and strip the Drain's own waits."""
    nops = [self.nc.sync.nop(nofuse=True) for _ in range(27)]
    drain_inst = self.nc.sync.drain()
    if drain_inst.ins.sync_info is None:
        drain_inst.ins.sync_info = mybir.SyncInfo(on_wait=[], on_update=[])
    wait_clock.add_sem_waits(
        drain_inst.ins, ScopedClock({None: tick_clock.global_clock})
    )
    waits = list(drain_inst.ins.sync_info.on_wait)
    assert len(waits) <= len(nops), f"{len(waits)} waits > {len(nops)} carriers"
    handles = {h.num: h for h in self.sems.allocated().values()}
    drain_inst.ins.sync_info.on_wait = []
    for nop, w in zip(nops, waits):
        nop.wait_op(handles[w.id], w.wait_value, "sem-ge")

    self.nc.all_engine_barrier()
    assert self.sems is not None
    popped = self.nc._tile_sem_poison_stack.pop()
    assert popped is self._sem_poison
    self.nc.clear_and_free_semaphores(list(self.sems.allocated().values()))
    self.nc.all_engine_barrier()


tile.TileContext._drain_and_barrier = _patched_drain_and_barrier

_MAX_WAITS = 1
_orig_commit_and_lower = tile.TileContext._commit_and_lower


def _patched_commit_and_lower(self, inst, original_block, old_bb_map, bb_to_exit_bb):
    """This walrus build encodes at most one sync wait per TPB instruction.
    Tile's scheduler attaches up to ~3; hoist the excess onto same-engine
    NoOp carriers emitted immediately before the instruction (equivalent
    semantics: the engine blocks on each wait in sequence)."""
    si = getattr(inst, "sync_info", None)
    if (
        si is not None
        and si.on_wait
        and len(si.on_wait) > _MAX_WAITS
        and inst.__class__.__name__.startswith("Inst")
    ):
        waits = list(si.on_wait)
        si.on_wait = waits[:_MAX_WAITS]
        for w in waits[_MAX_WAITS:]:
            carrier = mybir.InstNoOp(
                name=self.nc.get_next_instruction_name(),
                engine=inst.engine,
                sync_info=mybir.SyncInfo(on_wait=[w], on_update=[]),
                bass_nofuse=True,
            )
            self._commit_instruction(carrier)
    return _orig_commit_and_lower(self, inst, original_block, old_bb_map, bb_to_exit_bb)


tile.TileContext._commit_and_lower = _patched_commit_and_lower

_orig_tile_legalize = tile.tile_legalize


def _ldw_key(ins):
    try:
        ap = ins.ins[0]
        if getattr(ap, "kind", "") == "bass_symbolic_ap":
            bap = ap.bass_ap
            off = bap.offset
            if not isinstance(off, int):
                return None
            return ("sym", str(bap.tensor.name), off, str(ap.ap), str(ap.dtype))
        return (
            "phys",
            str(ap.memref),
            str(ap.memsetref),
            ap.offset,
            str(ap.ap),
            str(ap.dtype),
        )
    except Exception:
        return None


def _dedup_ldweights(ordered):
    """Drop an InstLdweights when the PE's weight registers already hold the
    same weights: identical (tensor, offset, pattern) as the previous
    Ldweights with only Matmults in between on the PE. The preceding
    identical Ldweights carries the same producer dependency, and matmuls
    consume the array-resident copy, so this is sync-safe. Pays off for the
    PV sweep where po0/po1/pn share one stationary P^T slice."""
    for bb, insts in ordered.items():
        last_key = None
        drop = set()
        for ins in insts:
            if str(getattr(ins, "engine", "")) != "EngineType.PE":
                continue
            tn = type(ins).__name__
            if tn == "InstLdweights":
                key = _ldw_key(ins)
                if (
                    key is not None
                    and key == last_key
                    and not (ins.sync_info and ins.sync_info.on_update)
                ):
                    drop.add(ins.name)
                else:
                    last_key = key
            elif tn == "InstMatmult":
                continue
            else:
                last_key = None
        if drop:
            ordered[bb] = [i for i in insts if i.name not in drop]
    return ordered


_REGROUP_WINDOW = 24


def _mm_out_ref(mm):
    try:
        return str(mm.outs[0].memref)
    except Exception:
        return None


def _regroup_pe_units(ordered):
    """Re-pair PE (Ldweights, Matmult) units that share the same stationary
    weights: if the scheduler splits the po0/po1/pn sweeps, the matmuls per
    P^T slice no longer sit adjacent and the weight reload can't be elided.
    Move a unit up next to the previous unit with the same weights key when
    it is within a small window and no crossed unit writes the same PSUM
    tensor (preserves per-accumulator ordering; runs before semaphore
    assignment, so waits are recomputed for the new order)."""
    for bb, insts in ordered.items():
        pe_idx = [
            i
            for i, ins in enumerate(insts)
            if str(getattr(ins, "engine", "")) == "EngineType.PE"
        ]
        pe_seq = [insts[i] for i in pe_idx]
        items = []  # ("unit", key, out_ref, [ldw, mm]) or ("other", inst)
        i = 0
        while i < len(pe_seq):
            ins = pe_seq[i]
            if (
                type(ins).__name__ == "InstLdweights"
                and i + 1 < len(pe_seq)
                and type(pe_seq[i + 1]).__name__ == "InstMatmult"
            ):
                mm = pe_seq[i + 1]
                items.append(("unit", _ldw_key(ins), _mm_out_ref(mm), [ins, mm]))
                i += 2
            else:
                items.append(("other", None, None, [ins]))
                i += 1
        out_items = []
        for it in items:
            kind, key, oref, _ = it
            if kind != "unit" or key is None:
                out_items.append(it)
                continue
            j = None
            for d in range(1, min(_REGROUP_WINDOW, len(out_items)) + 1):
                cand = out_items[-d]
                if cand[0] != "unit":
                    break
                if cand[1] == key:
                    j = len(out_items) - d
                    break
            if j is not None:
                crossed = out_items[j + 1 :]
                if all(c[2] != oref for c in crossed):
                    out_items.insert(j + 1, it)
                    continue
            out_items.append(it)
        new_pe_seq = [ins for it in out_items for ins in it[3]]
        assert len(new_pe_seq) == len(pe_seq)
        new_insts = list(insts)
        for pos, ins in zip(pe_idx, new_pe_seq):
            new_insts[pos] = ins
        ordered[bb] = new_insts
    return ordered


# Elides the pn rider's and po1's redundant weight reloads in the PV sweep.
# Consistently a small win in interleaved A/Bs (median -1.5us, best-wall
# -5us) and numerically bit-identical on the reference inputs.
_LDW_DEDUP_ENABLED = [True]


def _patched_tile_legalize(*args, **kwargs):
    out = _orig_tile_legalize(*args, **kwargs)
    if _LDW_DEDUP_ENABLED[0]:
        out = _dedup_ldweights(_regroup_pe_units(out))
    return out


tile.tile_legalize = _patched_tile_legalize


def build_nc(
    repeats: int = 1,
    hw_loop: bool = False,
    timing: bool = False,
    ndr: int | None = None,
    qk_only: bool = False,
    no_pn: bool = False,
    no_exp: bool = False,
    dma_off: bool = False,
    diag_first: bool = False,
    ps_o_bufs: int = 2,
    ps_s_bufs: int = 2,
    ps_n_bufs: int = 2,
    mask_mode: str = "dve",  # "dve" (psum add) | "gpsimd" (post-exp zero)
    # V-load emission point: "mid" = between the off-diagonal and diagonal
    # QK tiles (diagonal-QK window hides the V DMA without competing with
    # the chunk-start Q/K loads; measured best), True = at the PV section,
    # False = at chunk start alongside Q/K.
    v_late="mid",
    dma_split: bool = False,  # issue K loads on the ACT-engine DGE queues
    ktg_bufs: int = 5,
    qt_bufs: int = 2,
    v_bufs: int = 21,
    pt_bufs: int = 21,
) -> bass.Bass:
    if ndr is None:
        ndr = NDR
    d16 = D - 256 * ndr
    ndc16 = d16 // P  # fp16 d-chunks of 128
    nc = bass.Bass()
    kind_in = {} if timing else {"kind": "ExternalInput"}
    kind_out = {} if timing else {"kind": "ExternalOutput"}
    if d16:
        qT = nc.dram_tensor("qT", [BPC, d16, L], F16, **kind_in)
        kT = nc.dram_tensor("kT", [BPC, d16, L], F16, **kind_in)
    if ndr:
        q8 = nc.dram_tensor("q8", [BPC, 256 * ndr, L], F8, **kind_in)
        k8 = nc.dram_tensor("k8", [BPC, 256 * ndr, L], F8, **kind_in)
    v = nc.dram_tensor("v", [BPC, L, D], F16, **kind_in)
    mT = nc.dram_tensor("maskT", [P, P], F32, **kind_in)
    o = nc.dram_tensor("o", [BPC, L, D], F16, **kind_out)
    if timing:
        tin = nc.dram_tensor("tin", [1, 8], F32, kind="ExternalInput")
        tout = nc.dram_tensor("tout", [1, 8], F32, kind="ExternalOutput")

    with tile.TileContext(nc) as tc:
        with (
            tc.tile_pool(name="singles", bufs=1) as singles,
            tc.tile_pool(name="ktg", bufs=ktg_bufs) as ktg_pool,
            tc.tile_pool(name="k8g", bufs=ktg_bufs) as k8g_pool,
            tc.tile_pool(name="qtc", bufs=qt_bufs) as qt_pool,
            tc.tile_pool(name="vt", bufs=v_bufs) as v_pool,
            tc.tile_pool(name="pt", bufs=pt_bufs) as pt_pool,
            tc.tile_pool(name="outp", bufs=6) as out_pool,
            tc.tile_pool(name="smalls", bufs=6) as small_pool,
            tc.tile_pool(name="ps_s", bufs=ps_s_bufs, space="PSUM") as ps_s_pool,
            tc.tile_pool(name="ps_o", bufs=ps_o_bufs, space="PSUM") as ps_o_pool,
            tc.tile_pool(name="ps_n", bufs=ps_n_bufs, space="PSUM") as ps_n_pool,
        ):
            # maskT[k, q] = 0 if q >= k else MASK_VAL (S^T layout: partitions
            # are k, free dim is q) for the diagonal 128x128 blocks. Loaded
            # from DRAM (host constant) so cold-start needs no GPSIMD pass
            # in front of chunk 0's all-diagonal mask adds.
            maskT = singles.tile([P, P], F32)
            nc.sync.dma_start(out=maskT, in_=mT[:, :])
            ones16 = singles.tile([P, 1], F16)
            nc.vector.memset(ones16, 1.0)

            if timing:
                tt = singles.tile([1, 8], F32)
                nc.sync.dma_start(out=tt, in_=tin[:, :])
                nc.sync.dma_start(out=tout[:, :], in_=tt)

            def body():
                for b in range(BPC):
                    if d16:
                        ktv = kT[b].rearrange("(dc p) k -> p dc k", p=P)
                        qtv = qT[b].rearrange("(dc p) q -> p dc q", p=P)
                    if ndr:
                        k8v = k8[b].rearrange("(r h p) k -> p r h k", p=P, h=2)
                        q8v = q8[b].rearrange("(r h p) q -> p r h q", p=P, h=2)
                    vv = v[b].rearrange("(kt p) d -> p kt d", p=P)

                    kgs = {}  # k-group g covers key tiles 4g..4g+3
                    vts = {}
                    for qs in range(NQS):
                        qsl = slice(512 * qs, 512 * (qs + 1))
                        # fp16 q/k chunk loads, in dc-halves so the first
                        # matmuls can start after half the chunk has landed
                        if d16:
                            ha = ndc16 - ndc16 // 2  # first-half d-chunks
                            QTa = qt_pool.tile([P, ha, 512], F16, tag="qta")
                            kga = ktg_pool.tile([P, ha, 512], F16, tag="kga")
                            keng = nc.scalar if dma_split else nc.sync
                            if not dma_off:
                                if b == 0 and qs == 0:
                                    # cold start: per-d-chunk pieces so the
                                    # first matmul waits on 2x128KB, not
                                    # 2x384KB (subtile deps gate per piece)
                                    for dc in range(ha):
                                        nc.sync.dma_start(
                                            out=QTa[:, dc, :],
                                            in_=qtv[:, dc, qsl],
                                        )
                                        keng.dma_start(
                                            out=kga[:, dc, :],
                                            in_=ktv[:, dc, qsl],
                                        )
                                else:
                                    nc.sync.dma_start(out=QTa, in_=qtv[:, 0:ha, qsl])
                                    keng.dma_start(out=kga, in_=ktv[:, 0:ha, qsl])
                            if ndc16 // 2:
                                QTb = qt_pool.tile(
                                    [P, ndc16 // 2, 512], F16, tag="qtb"
                                )
                                kgb = ktg_pool.tile(
                                    [P, ndc16 // 2, 512], F16, tag="kgb"
                                )
                                if not dma_off:
                                    if b == 0 and qs == 0:
                                        for dc in range(ndc16 // 2):
                                            nc.sync.dma_start(
                                                out=QTb[:, dc, :],
                                                in_=qtv[:, ha + dc, qsl],
                                            )
                                            keng.dma_start(
                                                out=kgb[:, dc, :],
                                                in_=ktv[:, ha + dc, qsl],
                                            )
                                    else:
                                        nc.sync.dma_start(
                                            out=QTb, in_=qtv[:, ha:, qsl]
                                        )
                                        keng.dma_start(
                                            out=kgb, in_=ktv[:, ha:, qsl]
                                        )
                            else:
                                QTb = kgb = None
                        else:
                            ha = 0
                            QTa = QTb = kga = kgb = None
                        if ndr:
                            QT8 = qt_pool.tile([P, ndr, 2, 512], F8, tag="qt8")
                            kg8 = k8g_pool.tile([P, ndr, 2, 512], F8, tag="kg8")
                            if not dma_off:
                                nc.sync.dma_start(out=QT8, in_=q8v[:, :, :, qsl])
                                (nc.scalar if dma_split else nc.sync).dma_start(
                                    out=kg8, in_=k8v[:, :, :, qsl])
                        else:
                            QT8 = kg8 = None
                        kgs[qs] = (kga, kgb, kg8)

                        def load_v():
                            for kt in range(4 * qs, 4 * qs + 4):
                                vt = v_pool.tile([P, D], F16)
                                nc.sync.dma_start(out=vt, in_=vv[:, kt, :])
                                vts[kt] = vt

                        if not v_late:
                            load_v()

                        # ---- scores + exp for this 512-wide q chunk ----
                        # Off-diagonal tiles first: they use K groups already
                        # resident from earlier chunks, so the fresh K chunk
                        # DMA hides under them. V loads are emitted between
                        # the two groups ("mid"): the diagonal-QK window lets
                        # the V DMA land before PV needs the diagonal V tile,
                        # without competing with the chunk-start Q/K loads.
                        if diag_first:
                            kt_order = list(range(4 * qs, 4 * qs + 4)) + list(
                                range(0, 4 * qs)
                            )
                        else:
                            kt_order = list(range(4 * qs + 4))
                        pts = {}
                        v_loaded = False
                        for kt in kt_order:
                            if (
                                v_late == "mid"
                                and not v_loaded
                                and kt >= 4 * qs
                                and not qk_only
                            ):
                                load_v()
                                v_loaded = True
                            # first valid (unmasked) column within the chunk
                            q_lo = max(0, 128 * kt - 512 * qs)
                            pt = pt_pool.tile([P, 512], F16)
                            ps = ps_s_pool.tile([P, 512], F32)
                            cga, cgb, cg8 = kgs[kt // 4]
                            kcol = 128 * (kt % 4)
                            nmm = ndc16 + ndr
                            imm = 0
                            for dc in range(ndc16):
                                if dc < ha:
                                    kgt, qtt, dco = cga, QTa, dc
                                else:
                                    kgt, qtt, dco = cgb, QTb, dc - ha
                                nc.tensor.matmul(
                                    ps[:, q_lo:],
                                    lhsT=kgt[:, dco, kcol : kcol + P],
                                    rhs=qtt[:, dco, q_lo:],
                                    start=(imm == 0),
                                    stop=(imm == nmm - 1),
                                )
                                imm += 1
                            for r in range(ndr):
                                nc.tensor.matmul(
                                    ps[:, q_lo:],
                                    lhsT=cg8[:, r, :, kcol : kcol + P],
                                    rhs=QT8[:, r, :, q_lo:],
                                    start=(imm == 0),
                                    stop=(imm == nmm - 1),
                                    perf_mode=DRMODE,
                                )
                                imm += 1
                            if no_exp:
                                pts[kt] = pt
                                continue
                            # NOTE: pt[:, :q_lo] is intentionally left
                            # unwritten -- PV and the pn riders only read
                            # pt[:, 128*qtl:] with 128*qtl >= q_lo (the
                            # causality condition), so a memset there would
                            # be dead work sitting in the DVE FIFO right
                            # before the critical diagonal mask adds.
                            diag = kt >= 4 * qs
                            if diag and mask_mode == "dve":
                                # diagonal block: additive causal mask in PSUM
                                nc.vector.tensor_add(
                                    out=ps[:, q_lo : q_lo + P],
                                    in0=ps[:, q_lo : q_lo + P],
                                    in1=maskT,
                                )
                            nc.scalar.activation(
                                out=pt[:, q_lo:],
                                in_=ps[:, q_lo:],
                                func=mybir.ActivationFunctionType.Exp,
                                scale=SCALE,
                            )
                            if diag and mask_mode == "gpsimd":
                                # zero the masked (q < k) triangle of the
                                # diagonal 128x128 block after the exp on the
                                # otherwise idle GPSIMD engine: keeps the
                                # QK->exp PSUM chain free of the DVE hop and
                                # yields exactly the reference's exp(-1e10)=0.
                                nc.gpsimd.affine_select(
                                    out=pt[:, q_lo : q_lo + P],
                                    in_=pt[:, q_lo : q_lo + P],
                                    compare_op=mybir.AluOpType.is_ge,
                                    fill=0.0,
                                    base=0,
                                    channel_multiplier=-1,  # keep where q >= k
                                    pattern=[[1, P]],
                                )
                            pts[kt] = pt

                        if qk_only:
                            continue
                        if v_late and not v_loaded:
                            load_v()

                        # ---- probs @ V for the 4 q-tiles of this chunk ----
                        # Denominators ride the sweep as N=1 matmuls on the
                        # same stationary weights: pn[128q, 1] = sum_k P.
                        for qtl in range(4):
                            qt_g = 4 * qs + qtl
                            po0 = ps_o_pool.tile([P, 512], F32)
                            po1 = ps_o_pool.tile([P, 512], F32)
                            pn = ps_n_pool.tile([P, 1], F32)
                            for kt in range(qt_g + 1):
                                lh = pts[kt][:, 128 * qtl : 128 * (qtl + 1)]
                                first = kt == 0
                                last = kt == qt_g
                                nc.tensor.matmul(
                                    po0, lhsT=lh, rhs=vts[kt][:, 0:512],
                                    start=first, stop=last,
                                )
                                nc.tensor.matmul(
                                    po1, lhsT=lh, rhs=vts[kt][:, 512:1024],
                                    start=first, stop=last,
                                )
                                if not no_pn:
                                    nc.tensor.matmul(
                                        pn, lhsT=lh, rhs=ones16,
                                        start=first, stop=last,
                                    )
                            rec = small_pool.tile([P, 1], F32, tag="rec")
                            if no_pn:
                                nc.vector.memset(rec, 1.0)
                            else:
                                nc.vector.reciprocal(out=rec, in_=pn)
                            ot = out_pool.tile([P, D], F16)
                            nc.vector.tensor_scalar_mul(ot[:, 0:512], po0, rec)
                            nc.vector.tensor_scalar_mul(ot[:, 512:1024], po1, rec)
                            nc.sync.dma_start(
                                out=o[b, 128 * qt_g : 128 * (qt_g + 1), :],
                                in_=ot,
                            )

            if hw_loop and repeats > 1:
                with tc.For_i(0, repeats, 1):
                    body()
            else:
                for _ in range(repeats):
                    body()
    return nc


_NC_CACHE: dict = {}


def _get_nc(repeats: int = 1) -> bass.Bass:
    key = (repeats, NDR)
    if key not in _NC_CACHE:
        _NC_CACHE[key] = build_nc(repeats)
    return _NC_CACHE[key]


def make_in_maps(query: np.ndarray, key: np.ndarray, value: np.ndarray,
                 ndr: int | None = None):
    if ndr is None:
        ndr = NDR
    d16 = D - 256 * ndr
    try:
        import ml_dtypes

        f8 = ml_dtypes.float8_e4m3fn
    except ImportError:
        f8 = None
    maskT = np.where(
        np.arange(P)[None, :] >= np.arange(P)[:, None], 0.0, MASK_VAL
    ).astype(np.float32)
    in_maps = []
    for c in range(N_CORES):
        sl = slice(BPC * c, BPC * (c + 1))
        m = {"v": np.asarray(value[sl], dtype=np.float16), "maskT": maskT}
        qt = query[sl].transpose(0, 2, 1)  # [BPC, D, L]
        kt = key[sl].transpose(0, 2, 1)
        if d16:
            m["qT"] = np.ascontiguousarray(qt[:, :d16]).astype(np.float16)
            m["kT"] = np.ascontiguousarray(kt[:, :d16]).astype(np.float16)
        if ndr:
            m["q8"] = np.ascontiguousarray(qt[:, d16:]).astype(f8)
            m["k8"] = np.ascontiguousarray(kt[:, d16:]).astype(f8)
        in_maps.append(m)
    return in_maps


def kernel(query: np.ndarray, key: np.ndarray, value: np.ndarray) -> np.ndarray:
    query = np.asarray(query, dtype=np.float32)
    key = np.asarray(key, dtype=np.float32)
    value = np.asarray(value, dtype=np.float32)
    assert query.shape == (BPC * N_CORES, L, D), query.shape

    nc = _get_nc()
    res = run_bass_kernel_spmd(
        nc, make_in_maps(query, key, value), core_ids=list(range(N_CORES))
    )
    out = np.empty((BPC * N_CORES, L, D), dtype=np.float32)
    for c in range(N_CORES):
        out[BPC * c : BPC * (c + 1)] = np.asarray(
            res.results[c]["o"], dtype=np.float32
        )
    return out


# revision 34
# speedup vs baseline: 1.2136x; 1.2136x over previous
"""Causal attention kernel for Trainium2 (Bass/Tile), 8-core data parallel.

Problem: B=16, L=2048, D=1024 fp32.
    scores = q @ k^T  (per batch), causal additive mask (-1e10), softmax
    over keys with scale sqrt(1024)=32, out = probs @ v.

Sharding: batch dim across the 8 cores (2 batches per core), no
cross-core comms. Each core runs an identical program (SPMD).

Per-core scheme (per batch, matmuls fp16 in / fp32 PSUM; optionally the
last 256*ndr contraction dims of QK^T run as fp8-e4m3 DoubleRow pairs):
  - Host supplies q and k pre-transposed to [D, L] fp16 so the
    contraction dim D sits on SBUF partitions; v stays [L, D] fp16.
  - S^T tiles [128 k, 512 q] = K^T_chunk.T @ Q^T_chunk accumulated over
    d-chunks in PSUM. Off-diagonal k-tiles run first (they use K groups
    already resident from earlier chunks, hiding the fresh K chunk DMA;
    measured ~10us/iter better than diagonal-first).
  - P^T = exp((S^T + mask)/32) on ScalarE -> fp16 SBUF tiles. Fully
    masked column ranges are never computed NOR zeroed (PV's lhsT slices
    never touch them -- the causality condition guarantees it); the
    diagonal 128x128 sub-block gets a 0/-1e10 additive mask tile in PSUM.
  - O tile [128 q, 1024] = sum_kt P^T[kt].T @ V[kt] accumulated in PSUM.
    Softmax denominators ride the same sweep as N=1 matmuls
    (lhsT = the PV weights, rhs = ones) accumulating [128 q, 1] in
    PSUM -- per-partition sums that feed a DVE reciprocal directly, so
    the old [1,512] row-sum matmuls + transpose dance disappear.
  - Normalize with per-partition scalar multiply, DMA out fp16.
"""

import numpy as np

import concourse.bass as bass
import concourse.mybir as mybir
import concourse.tile as tile
from concourse.bass_utils import run_bass_kernel_spmd
from concourse.tile import ScopedClock

F32 = mybir.dt.float32
F16 = mybir.dt.float16
F8 = mybir.dt.float8e4
DRMODE = mybir.MatmulPerfMode.DoubleRow

N_CORES = 8
BPC = 2  # batches per core
L = 2048
D = 1024
P = 128
NQS = L // 512  # 4 q-chunks of 512
MASK_VAL = -1.0e10
SCALE = 1.0 / 32.0

NDR = 1  # DoubleRow fp8 d-steps (256 dims each) used for QK; rest fp16


def _patched_drain_and_barrier(self, tick_clock, wait_clock):
    """Workaround for walrus 'Too many sync wait commands' on the Tile exit
    Drain: re-emit the global-clock sem waits as standalone SP NoOps (one
    wait each) before the drain, and strip the Drain's own waits."""
    nops = [self.nc.sync.nop(nofuse=True) for _ in range(27)]
    drain_inst = self.nc.sync.drain()
    if drain_inst.ins.sync_info is None:
        drain_inst.ins.sync_info = mybir.SyncInfo(on_wait=[], on_update=[])
    wait_clock.add_sem_waits(
        drain_inst.ins, ScopedClock({None: tick_clock.global_clock})
    )
    waits = list(drain_inst.ins.sync_info.on_wait)
    assert len(waits) <= len(nops), f"{len(waits)} waits > {len(nops)} carriers"
    handles = {h.num: h for h in self.sems.allocated().values()}
    drain_inst.ins.sync_info.on_wait = []
    for nop, w in zip(nops, waits):
        nop.wait_op(handles[w.id], w.wait_value, "sem-ge")

    self.nc.all_engine_barrier()
    assert self.sems is not None
    popped = self.nc._tile_sem_poison_stack.pop()
    assert popped is self._sem_poison
    self.nc.clear_and_free_semaphores(list(self.sems.allocated().values()))
    self.nc.all_engine_barrier()


tile.TileContext._drain_and_barrier = _patched_drain_and_barrier

_MAX_WAITS = 1
_orig_commit_and_lower = tile.TileContext._commit_and_lower


def _patched_commit_and_lower(self, inst, original_block, old_bb_map, bb_to_exit_bb):
    """This walrus build encodes at most one sync wait per TPB instruction.
    Tile's scheduler attaches up to ~3; hoist the excess onto same-engine
    NoOp carriers emitted immediately before the instruction (equivalent
    semantics: the engine blocks on each wait in sequence)."""
    si = getattr(inst, "sync_info", None)
    if (
        si is not None
        and si.on_wait
        and len(si.on_wait) > _MAX_WAITS
        and inst.__class__.__name__.startswith("Inst")
    ):
        waits = list(si.on_wait)
        si.on_wait = waits[:_MAX_WAITS]
        for w in waits[_MAX_WAITS:]:
            carrier = mybir.InstNoOp(
                name=self.nc.get_next_instruction_name(),
                engine=inst.engine,
                sync_info=mybir.SyncInfo(on_wait=[w], on_update=[]),
                bass_nofuse=True,
            )
            self._commit_instruction(carrier)
    return _orig_commit_and_lower(self, inst, original_block, old_bb_map, bb_to_exit_bb)


tile.TileContext._commit_and_lower = _patched_commit_and_lower

_orig_tile_legalize = tile.tile_legalize


def _ldw_key(ins):
    try:
        ap = ins.ins[0]
        if getattr(ap, "kind", "") == "bass_symbolic_ap":
            bap = ap.bass_ap
            off = bap.offset
            if not isinstance(off, int):
                return None
            return ("sym", str(bap.tensor.name), off, str(ap.ap), str(ap.dtype))
        return (
            "phys",
            str(ap.memref),
            str(ap.memsetref),
            ap.offset,
            str(ap.ap),
            str(ap.dtype),
        )
    except Exception:
        return None


def _dedup_ldweights(ordered):
    """Drop an InstLdweights when the PE's weight registers already hold the
    same weights: identical (tensor, offset, pattern) as the previous
    Ldweights with only Matmults in between on the PE. The preceding
    identical Ldweights carries the same producer dependency, and matmuls
    consume the array-resident copy, so this is sync-safe. Pays off for the
    PV sweep where po0/po1/pn share one stationary P^T slice."""
    for bb, insts in ordered.items():
        last_key = None
        drop = set()
        for ins in insts:
            if str(getattr(ins, "engine", "")) != "EngineType.PE":
                continue
            tn = type(ins).__name__
            if tn == "InstLdweights":
                key = _ldw_key(ins)
                if (
                    key is not None
                    and key == last_key
                    and not (ins.sync_info and ins.sync_info.on_update)
                ):
                    drop.add(ins.name)
                else:
                    last_key = key
            elif tn == "InstMatmult":
                continue
            else:
                last_key = None
        if drop:
            ordered[bb] = [i for i in insts if i.name not in drop]
    return ordered


_REGROUP_WINDOW = 24


def _mm_out_ref(mm):
    try:
        return str(mm.outs[0].memref)
    except Exception:
        return None


def _regroup_pe_units(ordered):
    """Re-pair PE (Ldweights, Matmult) units that share the same stationary
    weights: if the scheduler splits the po0/po1/pn sweeps, the matmuls per
    P^T slice no longer sit adjacent and the weight reload can't be elided.
    Move a unit up next to the previous unit with the same weights key when
    it is within a small window and no crossed unit writes the same PSUM
    tensor (preserves per-accumulator ordering; runs before semaphore
    assignment, so waits are recomputed for the new order)."""
    for bb, insts in ordered.items():
        pe_idx = [
            i
            for i, ins in enumerate(insts)
            if str(getattr(ins, "engine", "")) == "EngineType.PE"
        ]
        pe_seq = [insts[i] for i in pe_idx]
        items = []  # ("unit", key, out_ref, [ldw, mm]) or ("other", inst)
        i = 0
        while i < len(pe_seq):
            ins = pe_seq[i]
            if (
                type(ins).__name__ == "InstLdweights"
                and i + 1 < len(pe_seq)
                and type(pe_seq[i + 1]).__name__ == "InstMatmult"
            ):
                mm = pe_seq[i + 1]
                items.append(("unit", _ldw_key(ins), _mm_out_ref(mm), [ins, mm]))
                i += 2
            else:
                items.append(("other", None, None, [ins]))
                i += 1
        out_items = []
        for it in items:
            kind, key, oref, _ = it
            if kind != "unit" or key is None:
                out_items.append(it)
                continue
            j = None
            for d in range(1, min(_REGROUP_WINDOW, len(out_items)) + 1):
                cand = out_items[-d]
                if cand[0] != "unit":
                    break
                if cand[1] == key:
                    j = len(out_items) - d
                    break
            if j is not None:
                crossed = out_items[j + 1 :]
                if all(c[2] != oref for c in crossed):
                    out_items.insert(j + 1, it)
                    continue
            out_items.append(it)
        new_pe_seq = [ins for it in out_items for ins in it[3]]
        assert len(new_pe_seq) == len(pe_seq)
        new_insts = list(insts)
        for pos, ins in zip(pe_idx, new_pe_seq):
            new_insts[pos] = ins
        ordered[bb] = new_insts
    return ordered


# Elides the pn rider's and po1's redundant weight reloads in the PV sweep.
# Consistently a small win in interleaved A/Bs (median -1.5us, best-wall
# -5us) and numerically bit-identical on the reference inputs.
_LDW_DEDUP_ENABLED = [True]


def _patched_tile_legalize(*args, **kwargs):
    out = _orig_tile_legalize(*args, **kwargs)
    if _LDW_DEDUP_ENABLED[0]:
        out = _dedup_ldweights(_regroup_pe_units(out))
    return out


tile.tile_legalize = _patched_tile_legalize


def build_nc(
    repeats: int = 1,
    hw_loop: bool = False,
    timing: bool = False,
    ndr: int | None = None,
    qk_only: bool = False,
    no_pn: bool = False,
    no_exp: bool = False,
    dma_off: bool = False,
    diag_first: bool = False,
    ps_o_bufs: int = 2,
    ps_s_bufs: int = 2,
    ps_n_bufs: int = 2,
    mask_mode: str = "dve",  # "dve" (psum add) | "gpsimd" (post-exp zero)
    # V-load emission point: "mid" = between the off-diagonal and diagonal
    # QK tiles (diagonal-QK window hides the V DMA without competing with
    # the chunk-start Q/K loads; measured best), True = at the PV section,
    # False = at chunk start alongside Q/K.
    v_late="mid",
    dma_split: bool = False,  # issue K loads on the ACT-engine DGE queues
    ktg_bufs: int = 5,
    qt_bufs: int = 2,
    v_bufs: int = 21,
    pt_bufs: int = 21,
) -> bass.Bass:
    if ndr is None:
        ndr = NDR
    d16 = D - 256 * ndr
    ndc16 = d16 // P  # fp16 d-chunks of 128
    nc = bass.Bass()
    kind_in = {} if timing else {"kind": "ExternalInput"}
    kind_out = {} if timing else {"kind": "ExternalOutput"}
    if d16:
        qT = nc.dram_tensor("qT", [BPC, d16, L], F16, **kind_in)
        kT = nc.dram_tensor("kT", [BPC, d16, L], F16, **kind_in)
    if ndr:
        q8 = nc.dram_tensor("q8", [BPC, 256 * ndr, L], F8, **kind_in)
        k8 = nc.dram_tensor("k8", [BPC, 256 * ndr, L], F8, **kind_in)
    v = nc.dram_tensor("v", [BPC, L, D], F16, **kind_in)
    mT = nc.dram_tensor("maskT", [P, P], F32, **kind_in)
    o = nc.dram_tensor("o", [BPC, L, D], F16, **kind_out)
    if timing:
        tin = nc.dram_tensor("tin", [1, 8], F32, kind="ExternalInput")
        tout = nc.dram_tensor("tout", [1, 8], F32, kind="ExternalOutput")

    with tile.TileContext(nc) as tc:
        with (
            tc.tile_pool(name="singles", bufs=1) as singles,
            tc.tile_pool(name="ktg", bufs=ktg_bufs) as ktg_pool,
            tc.tile_pool(name="k8g", bufs=ktg_bufs) as k8g_pool,
            tc.tile_pool(name="qtc", bufs=qt_bufs) as qt_pool,
            tc.tile_pool(name="vt", bufs=v_bufs) as v_pool,
            tc.tile_pool(name="pt", bufs=pt_bufs) as pt_pool,
            tc.tile_pool(name="outp", bufs=6) as out_pool,
            tc.tile_pool(name="smalls", bufs=6) as small_pool,
            tc.tile_pool(name="ps_s", bufs=ps_s_bufs, space="PSUM") as ps_s_pool,
            tc.tile_pool(name="ps_o", bufs=ps_o_bufs, space="PSUM") as ps_o_pool,
            tc.tile_pool(name="ps_n", bufs=ps_n_bufs, space="PSUM") as ps_n_pool,
        ):
            # maskT[k, q] = 0 if q >= k else MASK_VAL (S^T layout: partitions
            # are k, free dim is q) for the diagonal 128x128 blocks. Loaded
            # from DRAM (host constant) so cold-start needs no GPSIMD pass
            # in front of chunk 0's all-diagonal mask adds.
            maskT = singles.tile([P, P], F32)
            nc.sync.dma_start(out=maskT, in_=mT[:, :])
            ones16 = singles.tile([P, 1], F16)
            nc.vector.memset(ones16, 1.0)

            if timing:
                tt = singles.tile([1, 8], F32)
                nc.sync.dma_start(out=tt, in_=tin[:, :])
                nc.sync.dma_start(out=tout[:, :], in_=tt)

            def body():
                for b in range(BPC):
                    if d16:
                        ktv = kT[b].rearrange("(dc p) k -> p dc k", p=P)
                        qtv = qT[b].rearrange("(dc p) q -> p dc q", p=P)
                    if ndr:
                        k8v = k8[b].rearrange("(r h p) k -> p r h k", p=P, h=2)
                        q8v = q8[b].rearrange("(r h p) q -> p r h q", p=P, h=2)
                    vv = v[b].rearrange("(kt p) d -> p kt d", p=P)

                    kgs = {}  # k-group g covers key tiles 4g..4g+3
                    vts = {}
                    for qs in range(NQS):
                        qsl = slice(512 * qs, 512 * (qs + 1))
                        # fp16 q/k chunk loads, in dc-halves so the first
                        # matmuls can start after half the chunk has landed
                        if d16:
                            ha = ndc16 - ndc16 // 2  # first-half d-chunks
                            QTa = qt_pool.tile([P, ha, 512], F16, tag="qta")
                            kga = ktg_pool.tile([P, ha, 512], F16, tag="kga")
                            keng = nc.scalar if dma_split else nc.sync
                            if not dma_off:
                                if b == 0 and qs == 0:
                                    # cold start: per-d-chunk pieces so the
                                    # first matmul waits on 2x128KB, not
                                    # 2x384KB (subtile deps gate per piece)
                                    for dc in range(ha):
                                        nc.sync.dma_start(
                                            out=QTa[:, dc, :],
                                            in_=qtv[:, dc, qsl],
                                        )
                                        keng.dma_start(
                                            out=kga[:, dc, :],
                                            in_=ktv[:, dc, qsl],
                                        )
                                else:
                                    nc.sync.dma_start(out=QTa, in_=qtv[:, 0:ha, qsl])
                                    keng.dma_start(out=kga, in_=ktv[:, 0:ha, qsl])
                            if ndc16 // 2:
                                QTb = qt_pool.tile(
                                    [P, ndc16 // 2, 512], F16, tag="qtb"
                                )
                                kgb = ktg_pool.tile(
                                    [P, ndc16 // 2, 512], F16, tag="kgb"
                                )
                                if not dma_off:
                                    if b == 0 and qs == 0:
                                        for dc in range(ndc16 // 2):
                                            nc.sync.dma_start(
                                                out=QTb[:, dc, :],
                                                in_=qtv[:, ha + dc, qsl],
                                            )
                                            keng.dma_start(
                                                out=kgb[:, dc, :],
                                                in_=ktv[:, ha + dc, qsl],
                                            )
                                    else:
                                        nc.sync.dma_start(
                                            out=QTb, in_=qtv[:, ha:, qsl]
                                        )
                                        keng.dma_start(
                                            out=kgb, in_=ktv[:, ha:, qsl]
                                        )
                            else:
                                QTb = kgb = None
                        else:
                            ha = 0
                            QTa = QTb = kga = kgb = None
                        if ndr:
                            QT8 = qt_pool.tile([P, ndr, 2, 512], F8, tag="qt8")
                            kg8 = k8g_pool.tile([P, ndr, 2, 512], F8, tag="kg8")
                            if not dma_off:
                                nc.sync.dma_start(out=QT8, in_=q8v[:, :, :, qsl])
                                (nc.scalar if dma_split else nc.sync).dma_start(
                                    out=kg8, in_=k8v[:, :, :, qsl])
                        else:
                            QT8 = kg8 = None
                        kgs[qs] = (kga, kgb, kg8)

                        def load_v():
                            for kt in range(4 * qs, 4 * qs + 4):
                                vt = v_pool.tile([P, D], F16)
                                nc.sync.dma_start(out=vt, in_=vv[:, kt, :])
                                vts[kt] = vt

                        if not v_late:
                            load_v()

                        # ---- scores + exp for this 512-wide q chunk ----
                        # Off-diagonal tiles first: they use K groups already
                        # resident from earlier chunks, so the fresh K chunk
                        # DMA hides under them. V loads are emitted between
                        # the two groups ("mid"): the diagonal-QK window lets
                        # the V DMA land before PV needs the diagonal V tile,
                        # without competing with the chunk-start Q/K loads.
                        if diag_first:
                            kt_order = list(range(4 * qs, 4 * qs + 4)) + list(
                                range(0, 4 * qs)
                            )
                        else:
                            kt_order = list(range(4 * qs + 4))
                        pts = {}
                        v_loaded = False
                        for kt in kt_order:
                            if (
                                v_late == "mid"
                                and not v_loaded
                                and kt >= 4 * qs
                                and not qk_only
                            ):
                                load_v()
                                v_loaded = True
                            # first valid (unmasked) column within the chunk
                            q_lo = max(0, 128 * kt - 512 * qs)
                            pt = pt_pool.tile([P, 512], F16)
                            ps = ps_s_pool.tile([P, 512], F32)
                            cga, cgb, cg8 = kgs[kt // 4]
                            kcol = 128 * (kt % 4)
                            nmm = ndc16 + ndr
                            imm = 0
                            for dc in range(ndc16):
                                if dc < ha:
                                    kgt, qtt, dco = cga, QTa, dc
                                else:
                                    kgt, qtt, dco = cgb, QTb, dc - ha
                                nc.tensor.matmul(
                                    ps[:, q_lo:],
                                    lhsT=kgt[:, dco, kcol : kcol + P],
                                    rhs=qtt[:, dco, q_lo:],
                                    start=(imm == 0),
                                    stop=(imm == nmm - 1),
                                )
                                imm += 1
                            for r in range(ndr):
                                nc.tensor.matmul(
                                    ps[:, q_lo:],
                                    lhsT=cg8[:, r, :, kcol : kcol + P],
                                    rhs=QT8[:, r, :, q_lo:],
                                    start=(imm == 0),
                                    stop=(imm == nmm - 1),
                                    perf_mode=DRMODE,
                                )
                                imm += 1
                            if no_exp:
                                pts[kt] = pt
                                continue
                            # NOTE: pt[:, :q_lo] is intentionally left
                            # unwritten -- PV and the pn riders only read
                            # pt[:, 128*qtl:] with 128*qtl >= q_lo (the
                            # causality condition), so a memset there would
                            # be dead work sitting in the DVE FIFO right
                            # before the critical diagonal mask adds.
                            diag = kt >= 4 * qs
                            if diag and mask_mode == "dve":
                                # diagonal block: additive causal mask in PSUM
                                nc.vector.tensor_add(
                                    out=ps[:, q_lo : q_lo + P],
                                    in0=ps[:, q_lo : q_lo + P],
                                    in1=maskT,
                                )
                            nc.scalar.activation(
                                out=pt[:, q_lo:],
                                in_=ps[:, q_lo:],
                                func=mybir.ActivationFunctionType.Exp,
                                scale=SCALE,
                            )
                            if diag and mask_mode == "gpsimd":
                                # zero the masked (q < k) triangle of the
                                # diagonal 128x128 block after the exp on the
                                # otherwise idle GPSIMD engine: keeps the
                                # QK->exp PSUM chain free of the DVE hop and
                                # yields exactly the reference's exp(-1e10)=0.
                                nc.gpsimd.affine_select(
                                    out=pt[:, q_lo : q_lo + P],
                                    in_=pt[:, q_lo : q_lo + P],
                                    compare_op=mybir.AluOpType.is_ge,
                                    fill=0.0,
                                    base=0,
                                    channel_multiplier=-1,  # keep where q >= k
                                    pattern=[[1, P]],
                                )
                            pts[kt] = pt

                        if qk_only:
                            continue
                        if v_late and not v_loaded:
                            load_v()

                        # ---- probs @ V for the 4 q-tiles of this chunk ----
                        # Denominators ride the sweep as N=1 matmuls on the
                        # same stationary weights: pn[128q, 1] = sum_k P.
                        for qtl in range(4):
                            qt_g = 4 * qs + qtl
                            po0 = ps_o_pool.tile([P, 512], F32)
                            po1 = ps_o_pool.tile([P, 512], F32)
                            pn = ps_n_pool.tile([P, 1], F32)
                            for kt in range(qt_g + 1):
                                lh = pts[kt][:, 128 * qtl : 128 * (qtl + 1)]
                                first = kt == 0
                                last = kt == qt_g
                                nc.tensor.matmul(
                                    po0, lhsT=lh, rhs=vts[kt][:, 0:512],
                                    start=first, stop=last,
                                )
                                nc.tensor.matmul(
                                    po1, lhsT=lh, rhs=vts[kt][:, 512:1024],
                                    start=first, stop=last,
                                )
                                if not no_pn:
                                    nc.tensor.matmul(
                                        pn, lhsT=lh, rhs=ones16,
                                        start=first, stop=last,
                                    )
                            rec = small_pool.tile([P, 1], F32, tag="rec")
                            if no_pn:
                                nc.vector.memset(rec, 1.0)
                            else:
                                nc.vector.reciprocal(out=rec, in_=pn)
                            ot = out_pool.tile([P, D], F16)
                            nc.vector.tensor_scalar_mul(ot[:, 0:512], po0, rec)
                            nc.vector.tensor_scalar_mul(ot[:, 512:1024], po1, rec)
                            nc.sync.dma_start(
                                out=o[b, 128 * qt_g : 128 * (qt_g + 1), :],
                                in_=ot,
                            )

            if hw_loop and repeats > 1:
                with tc.For_i(0, repeats, 1):
                    body()
            else:
                for _ in range(repeats):
                    body()
    return nc


_NC_CACHE: dict = {}


def _get_nc(repeats: int = 1) -> bass.Bass:
    key = (repeats, NDR)
    if key not in _NC_CACHE:
        _NC_CACHE[key] = build_nc(repeats)
    return _NC_CACHE[key]


def make_in_maps(query: np.ndarray, key: np.ndarray, value: np.ndarray,
                 ndr: int | None = None):
    if ndr is None:
        ndr = NDR
    d16 = D - 256 * ndr
    try:
        import ml_dtypes

        f8 = ml_dtypes.float8_e4m3fn
    except ImportError:
        f8 = None
    maskT = np.where(
        np.arange(P)[None, :] >= np.arange(P)[:, None], 0.0, MASK_VAL
    ).astype(np.float32)
    in_maps = []
    for c in range(N_CORES):
        sl = slice(BPC * c, BPC * (c + 1))
        m = {"v": np.asarray(value[sl], dtype=np.float16), "maskT": maskT}
        qt = query[sl].transpose(0, 2, 1)  # [BPC, D, L]
        kt = key[sl].transpose(0, 2, 1)
        if d16:
            m["qT"] = np.ascontiguousarray(qt[:, :d16]).astype(np.float16)
            m["kT"] = np.ascontiguousarray(kt[:, :d16]).astype(np.float16)
        if ndr:
            m["q8"] = np.ascontiguousarray(qt[:, d16:]).astype(f8)
            m["k8"] = np.ascontiguousarray(kt[:, d16:]).astype(f8)
        in_maps.append(m)
    return in_maps


def kernel(query: np.ndarray, key: np.ndarray, value: np.ndarray) -> np.ndarray:
    query = np.asarray(query, dtype=np.float32)
    key = np.asarray(key, dtype=np.float32)
    value = np.asarray(value, dtype=np.float32)
    assert query.shape == (BPC * N_CORES, L, D), query.shape

    nc = _get_nc()
    res = run_bass_kernel_spmd(
        nc, make_in_maps(query, key, value), core_ids=list(range(N_CORES))
    )
    out = np.empty((BPC * N_CORES, L, D), dtype=np.float32)
    for c in range(N_CORES):
        out[BPC * c : BPC * (c + 1)] = np.asarray(
            res.results[c]["o"], dtype=np.float32
        )
    return out
